# revision 14
# baseline (speedup 1.0000x reference)
"""Multi-head attention (B=16, N=1024, C=384, H=6, D=64) on 8 trn2 cores.

Sharding: data-parallel over batch — each core computes 2 full batches.

Per-core layout strategy (all on one NeuronCore, f32 I/O):
  - x is loaded naturally [n, c] and transposed on PE to xT [c, n].
  - qkv^T = w_qkv.T @ x computed with q,k transposed (d on partitions) and
    v natural [n, d]; biases folded into the PSUM->SBUF copies.
  - S^T[k, q] = k^T.T @ q^T per head (K=64 contraction). The two 512-wide
    q-chunks run concurrently in the PE array via row tiling (rows 0-63 /
    64-127), using DMA-duplicated copies of q^T/k^T in the upper partitions.
  - exp on ScalarE with scale=1/8 folded in, PSUM->SBUF, bf16 out (P^T).
  - PV with an augmented [V_h | 1] stationary tensor: row 64 of the output
    is the softmax denominator, at no extra PE cost.
  - normalize: in-place reciprocal of the sum row, K=1 matmul broadcasts it
    across partitions into PSUM, row-scale fused into the copy to attnT.
  - proj: out[n, c2] = attnT.T @ w_proj + b_proj, streamed out per n-tile.

Stage emission order is A0 B0 C0 A1 B1 C1(+D0 interleaved) D1 so batch 0's
projection fills PE gaps during batch 1's ScalarE-bound attention, and PSUM
pool rotation chains never couple attention to the next batch's staging.
"""

import numpy as np
from contextlib import ExitStack, nullcontext

import concourse.bass as bass
import concourse.mybir as mybir
import concourse.tile as tile
from concourse import bacc
from concourse.bass_utils import run_bass_kernel_spmd
from concourse.masks import make_identity

f32 = mybir.dt.float32
f32r = mybir.dt.float32r
bf16 = mybir.dt.bfloat16
EXP = mybir.ActivationFunctionType.Exp

B, N, C = 16, 1024, 384
H, D = 6, 64
NCORES = 8
BL = B // NCORES           # batches per core
HP = H // 2                # head pairs
SCALE = D ** -0.5
P = 128
NT = N // P                # 8 n-tiles
CT = C // P                # 3 c-tiles
KT = N // P                # 8 k-tiles in attention
QC = 2                     # 512-wide q chunks
QW = N // QC               # 512


def _r(ap, dt=f32r):
    return ap.bitcast(dt)


def build_nc(repeat=1, hwloop=False):
    nc = bacc.Bacc("TRN2", target_bir_lowering=False, debug=False)

    x_d = nc.dram_tensor("x", [BL, N, C], f32, kind="ExternalInput").ap()
    wqkv_d = nc.dram_tensor("w_qkv", [C, 3 * C], f32, kind="ExternalInput").ap()
    bqkv_d = nc.dram_tensor("b_qkv", [3 * C], f32, kind="ExternalInput").ap()
    wproj_d = nc.dram_tensor("w_proj", [C, C], f32, kind="ExternalInput").ap()
    bproj_d = nc.dram_tensor("b_proj", [C], f32, kind="ExternalInput").ap()
    out_d = nc.dram_tensor("out", [BL, N, C], f32, kind="ExternalOutput").ap()

    with tile.TileContext(nc) as tc, ExitStack() as ctx:
        consts = ctx.enter_context(tc.tile_pool(name="consts", bufs=1))
        big = ctx.enter_context(tc.tile_pool(name="big", bufs=1))
        work4 = ctx.enter_context(tc.tile_pool(name="work4", bufs=4))
        db = ctx.enter_context(tc.tile_pool(name="db", bufs=2))
        ps_st = ctx.enter_context(tc.tile_pool(name="ps_st", bufs=2, space="PSUM"))
        ps_pv = ctx.enter_context(tc.tile_pool(name="ps_pv", bufs=2, space="PSUM"))
        ps_wk = ctx.enter_context(tc.tile_pool(name="ps_wk", bufs=2, space="PSUM"))

        # ---- constants ----
        ident = consts.tile([P, P], f32)
        make_identity(nc, ident)
        ones64 = consts.tile([P, 64], f32)
        nc.vector.memset(ones64[:], 1.0)

        wqkv_raw = big.tile([P, CT, 3 * C], f32, tag="attnT")
        for kt in range(CT):
            for h2 in range(4):
                nc.sync.dma_start(
                    wqkv_raw[:, kt, h2 * 288:(h2 + 1) * 288],
                    wqkv_d.rearrange("(kt p) m -> p kt m", p=P)[
                        :, kt, h2 * 288:(h2 + 1) * 288],
                )
        wqkv_sb = consts.tile([P, CT, 3 * C], bf16)
        nc.vector.tensor_copy(wqkv_sb[:], wqkv_raw[:])
        wproj_raw = db.tile([P, CT, C], f32, tag="v_sb")
        for kt in range(CT):
            nc.sync.dma_start(
                wproj_raw[:, kt, :],
                wproj_d.rearrange("(kt p) m -> p kt m", p=P)[:, kt, :],
            )
        wproj_sb = consts.tile([P, CT, C], bf16)
        nc.vector.tensor_copy(wproj_sb[:], wproj_raw[:])
        # per-partition bias for the 6 qk c'-tiles
        bqk_sb = consts.tile([P, 6], f32)
        nc.sync.dma_start(bqk_sb[:], bqkv_d[0:768].rearrange("(t p) -> p t", p=P))
        # broadcast biases (vary along free dim)
        bv_sb = consts.tile([P, C], f32)
        nc.sync.dma_start(bv_sb[:], bqkv_d[None, 768:1152].to_broadcast((P, C)))
        bp_sb = consts.tile([P, C], f32)
        nc.sync.dma_start(bp_sb[:], bproj_d[None, :].to_broadcast((P, C)))

        def stage_ab(b):
            """Load x, transpose to xT, compute qkT and the augmented v."""
            xb = big.tile([P, NT, C], f32, tag="xb")
            for nt in range(NT):
                nc.sync.dma_start(
                    xb[:, nt, :],
                    x_d[b].rearrange("(t p) c -> p t c", p=P)[:, nt, :],
                )
            xT = db.tile([P, CT, N], bf16, tag="xT")
            for half in range(2):
                for ct in range(CT):
                    g = ps_wk.tile([P, QW], f32, tag="wk")
                    for j in range(4):
                        nt = half * 4 + j
                        nc.tensor.transpose(
                            g[:, j * P:(j + 1) * P],
                            xb[:, nt, ct * P:(ct + 1) * P],
                            ident[:],
                        )
                    nc.vector.tensor_copy(
                        xT[:, ct, half * QW:(half + 1) * QW], g[:])

            # hp0's q,k first so attention can begin while the rest finishes
            qkT = db.tile([P, 6, N], bf16, tag="qkT")
            for m in (0, 3, 1, 4, 2, 5):
                for ch in range(QC):
                    ps = ps_wk.tile([P, QW], f32, tag="wk")
                    for kt in range(CT):
                        nc.tensor.matmul(
                            ps[:],
                            lhsT=wqkv_sb[:, kt, m * P:(m + 1) * P],
                            rhs=xT[:, kt, ch * QW:(ch + 1) * QW],
                            start=(kt == 0), stop=(kt == CT - 1),
                        )
                    nc.vector.tensor_scalar_add(
                        qkT[:, m, ch * QW:(ch + 1) * QW], ps[:],
                        bqk_sb[:, m:m + 1])
            # v natural [n, (h [d|1])] in bf16, augmented with a ones column
            # per head so the PV matmul emits softmax denominators in row 64
            v_sb = db.tile([P, NT, H * (D + 1)], bf16, tag="v_sb")
            nc.vector.memset(
                v_sb[:].rearrange("p t (h e) -> p t h e", e=D + 1)[:, :, :, D:],
                1.0)
            for nt in range(NT):
                ps = ps_wk.tile([P, QW], f32, tag="wk")
                for kt in range(CT):
                    nc.tensor.matmul(
                        ps[:, 0:C],
                        lhsT=xT[:, kt, nt * P:(nt + 1) * P],
                        rhs=wqkv_sb[:, kt, 768:1152],
                        start=(kt == 0), stop=(kt == CT - 1),
                    )
                nc.vector.tensor_tensor(
                    v_sb[:, nt].rearrange("p (h e) -> p h e", e=D + 1)[:, :, 0:D],
                    ps[:, 0:C].rearrange("p (h e) -> p h e", e=D),
                    bv_sb[:].rearrange("p (h e) -> p h e", e=D),
                    mybir.AluOpType.add,
                )
            return qkT, v_sb

        def stage_c_hp(qkT, v_sb, attnT, hp):
            """One head pair of attention: S^T, exp, PV(+sums), normalize.

            The pair's two heads live in disjoint partition halves of qkT
            (d-dims 0-63 / 64-127), so their S^T matmuls run concurrently in
            disjoint PE row groups with no operand duplication: head 0 writes
            st cols 0:N, head 1 writes cols N:2N (different bf16 PSUM banks).
            One exp call then covers both heads' scores for the k-tile.
            """
            pair_aus = {}
            pt = big.tile([P, KT, QC, 2, QW], bf16, tag="pt")
            for kt in range(KT):
                for ch in range(QC):
                    st = ps_st.tile([P, N], f32, tag="st")
                    nc.tensor.matmul(
                        st[:, 0:QW],
                        lhsT=qkT[0:64, 3 + hp, kt * P:(kt + 1) * P],
                        rhs=qkT[0:64, hp, ch * QW:(ch + 1) * QW],
                        tile_position=(0, 0), start=True, stop=True,
                    )
                    nc.tensor.matmul(
                        st[:, QW:N],
                        lhsT=qkT[64:128, 3 + hp, kt * P:(kt + 1) * P],
                        rhs=qkT[64:128, hp, ch * QW:(ch + 1) * QW],
                        tile_position=(64, 0), start=True, stop=True,
                    )
                    nc.scalar.activation(
                        pt[:, kt, ch, :, :], st[:], EXP, scale=SCALE)

            for head_i in (0, 1):
                head = 2 * hp + head_i
                # PV with the augmented [V_h | 1] stationary tensor
                au = work4.tile([65, N], f32r, tag="attnU")
                for ch in range(QC):
                    po = ps_pv.tile([65, QW], f32, tag="pv")
                    for kt in range(KT):
                        nc.tensor.matmul(
                            po[:],
                            lhsT=v_sb[:, kt,
                                      head * (D + 1):(head + 1) * (D + 1)],
                            rhs=pt[:, kt, ch, head_i, :],
                            start=(kt == 0), stop=(kt == KT - 1),
                        )
                    nc.vector.tensor_copy(
                        au[:, ch * QW:(ch + 1) * QW], po[0:65, :])
                pair_aus[head_i] = au

            # normalize: in-place reciprocal of the sum row, broadcast to 64
            # partitions with a K=1 matmul (in the pv pool), multiply
            for head_i, base in ((0, 0), (1, 64)):
                au = pair_aus[head_i]
                with nc.allow_low_precision(
                        reason="f32r rounding of softmax recip"):
                    nc.vector.reciprocal(au[64:65, :], au[64:65, :])
                if head_i == 0:
                    dst = attnT[0:64, hp, :]
                else:
                    an = db.tile([64, N], bf16, tag="attnN")
                    dst = an[:]
                for ch in range(QC):
                    rb = ps_pv.tile([P, QW], f32, tag="pv")
                    nc.tensor.matmul(
                        rb[0:64, :],
                        lhsT=_r(ones64[64:65, :]),
                        rhs=_r(au[64:65, ch * QW:(ch + 1) * QW]),
                        tile_position=(64, 0),
                        start=True, stop=True,
                    )
                    nc.vector.tensor_mul(
                        dst[:, ch * QW:(ch + 1) * QW],
                        au[0:64, ch * QW:(ch + 1) * QW],
                        rb[0:64, :],
                    )
                if head_i == 1:
                    nc.sync.dma_start(attnT[64:128, hp, :], an[:])

        def stage_d(attnT, b, nts):
            for nt in nts:
                ps = ps_wk.tile([P, QW], f32, tag="wk")
                for ct in range(CT):
                    nc.tensor.matmul(
                        ps[:, 0:C],
                        lhsT=attnT[:, ct, nt * P:(nt + 1) * P],
                        rhs=wproj_sb[:, ct, :],
                        start=(ct == 0), stop=(ct == CT - 1),
                    )
                ob = db.tile([P, C], f32, tag="ob", bufs=4)
                nc.vector.tensor_add(ob[:], ps[:, 0:C], bp_sb[:])
                nc.sync.dma_start(
                    out_d[b].rearrange("(t p) c -> p t c", p=P)[:, nt, :],
                    ob[:],
                )

        loop_ctx = tc.For_i(0, repeat, 1) if hwloop else nullcontext(None)
        with loop_ctx:
            for rep in range(1 if hwloop else repeat):
                qkT0, v0 = stage_ab(0)
                attnT0 = big.tile([P, HP, N], bf16, tag="attnT")
                for hp in range(HP):
                    stage_c_hp(qkT0, v0, attnT0, hp)
                qkT1, v1 = stage_ab(1)
                attnT1 = big.tile([P, HP, N], bf16, tag="attnT")
                # interleave batch0's projection into batch1's attention so
                # PE has dense work while ScalarE runs the exps
                d0_sched = [range(NT), (), ()]
                for hp in range(HP):
                    stage_c_hp(qkT1, v1, attnT1, hp)
                    stage_d(attnT0, 0, d0_sched[hp])
                stage_d(attnT1, 1, range(NT))

    nc.compile()
    return nc


_NC_CACHE = {}


def _get_nc():
    if "nc" not in _NC_CACHE:
        _NC_CACHE["nc"] = build_nc()
    return _NC_CACHE["nc"]


def kernel(x, w_qkv, b_qkv, w_proj, b_proj):
    x = np.asarray(x, dtype=np.float32)
    w_qkv = np.asarray(w_qkv, dtype=np.float32)
    b_qkv = np.asarray(b_qkv, dtype=np.float32)
    w_proj = np.asarray(w_proj, dtype=np.float32)
    b_proj = np.asarray(b_proj, dtype=np.float32)

    nc = _get_nc()
    in_maps = [
        {
            "x": np.ascontiguousarray(x[i * BL:(i + 1) * BL]),
            "w_qkv": w_qkv,
            "b_qkv": b_qkv,
            "w_proj": w_proj,
            "b_proj": b_proj,
        }
        for i in range(NCORES)
    ]
    res = run_bass_kernel_spmd(nc, in_maps, list(range(NCORES)))
    return np.concatenate([res.results[i]["out"] for i in range(NCORES)], axis=0)

